# revision 14
# baseline (speedup 1.0000x reference)
"""Causal multi-head attention block (QKV proj -> causal attn -> out proj)
for Trainium2, sharded over 8 NeuronCores: data-parallel over batch (B=2),
tensor-parallel over heads (16 heads -> 4 heads per core).

Per-core device program (SPMD, same NEFF, different data):
  inputs : xT [1024,2048] (x[b].T), wq/wk/wv [1024,256], wp [256,1024],
           bq/bk/bv [256]
  outputs: outp [2048,1024] (partial out-proj, host sums 4 cores per batch),
           kout/vout [2048,256] (k,v natural layout for `present`)

Everything on chip is computed in a "transposed" layout so no input
transpose is ever needed on device:
  qT,kT [256,2048] = W.T @ x.T   (Dh on partitions -> scores need no transpose)
  V      [2048,256] = x @ Wv     (natural; used as PV stationary with a fused
                                  ones-column so PV also emits softmax denom)
  scoresT[s_k, s_q] tiles -> exp (no max subtraction; values are bounded)
  attnT  [256,2048] -> out proj natural [2048,1024]
"""

import os
import numpy as np

import concourse.bass as bass
import concourse.bacc as bacc
import concourse.mybir as mybir
import concourse.tile as tile

S = 2048
D = 1024
H = 16
Dh = 64
HPC = 4            # heads per core
F = HPC * Dh       # 256 channels per core
NK = D // 128      # 8 contraction chunks over D
NSK = S // 128     # 16 key tiles
NSQ = S // 512     # 4 query chunks
N_CORES = 8

f32 = mybir.dt.float32
f32r = mybir.dt.float32r
EXP = mybir.ActivationFunctionType.Exp
IDENT = mybir.ActivationFunctionType.Identity

# stash for test harness: exec_time_ns of the last traced run
LAST_EXEC_TIME_NS = None
LAST_RESULTS = None


def round_f32r(a):
    """Round fp32 to the PE's fp32r format (11-bit mantissa, round-half-up
    at bit 12) - matches walrus cast_fp32_to_fp32r."""
    u = np.ascontiguousarray(a, dtype=np.float32).view(np.uint32)
    r = ((u.astype(np.uint64) + 0x800) & 0xFFFFF000).astype(np.uint32)
    return r.view(np.float32)


def build_nc():
    nc = bacc.Bacc("TRN2", target_bir_lowering=False, debug=False)

    xT = nc.dram_tensor("xT", [D, S], f32r, kind="ExternalInput").ap()
    wq = nc.dram_tensor("wq", [D, F], f32r, kind="ExternalInput").ap()
    wk = nc.dram_tensor("wk", [D, F], f32r, kind="ExternalInput").ap()
    wv = nc.dram_tensor("wv", [D, F], f32r, kind="ExternalInput").ap()
    wp = nc.dram_tensor("wp", [F, D], f32r, kind="ExternalInput").ap()
    bq = nc.dram_tensor("bq", [F], f32, kind="ExternalInput").ap()
    bk = nc.dram_tensor("bk", [F], f32, kind="ExternalInput").ap()
    bv = nc.dram_tensor("bv", [F], f32, kind="ExternalInput").ap()

    outp = nc.dram_tensor("outp", [S, D], f32, kind="ExternalOutput").ap()
    kout = nc.dram_tensor("kout", [S, F], f32, kind="ExternalOutput").ap()
    vout = nc.dram_tensor("vout", [S, F], f32, kind="ExternalOutput").ap()

    with tile.TileContext(nc) as tc:
        _body(nc, tc, xT, wq, wk, wv, wp, bq, bk, bv, outp, kout, vout)

    nc.compile()
    return nc


def _body(nc, tc, xT, wq, wk, wv, wp, bq, bk, bv, outp, kout, vout):
    from contextlib import ExitStack

    ctx = ExitStack()
    with ctx:
        wpool = ctx.enter_context(tc.tile_pool(name="wpool", bufs=1))
        xpool = ctx.enter_context(tc.tile_pool(name="xpool", bufs=32))
        qkpool = ctx.enter_context(tc.tile_pool(name="qkpool", bufs=2))
        vpool = ctx.enter_context(tc.tile_pool(name="vpool", bufs=16))
        knpool = ctx.enter_context(tc.tile_pool(name="knpool", bufs=3))
        apool = ctx.enter_context(tc.tile_pool(name="apool", bufs=2))
        stpool = ctx.enter_context(tc.tile_pool(name="stpool", bufs=8))
        prpool = ctx.enter_context(tc.tile_pool(name="prpool", bufs=4))
        smpool = ctx.enter_context(tc.tile_pool(name="smpool", bufs=2))
        cpool = ctx.enter_context(tc.tile_pool(name="cpool", bufs=1))
        psum = ctx.enter_context(tc.tile_pool(name="psum", bufs=4, space="PSUM"))

        # ---- weights for q/k first: QKV starts as soon as x chunk 0 lands
        wq_sb = wpool.tile([128, NK, F], f32r, name="wq_sb")
        wk_sb = wpool.tile([128, NK, F], f32r, name="wk_sb")
        nc.sync.dma_start(wq_sb, wq.rearrange("(c p) n -> p c n", p=128))
        nc.sync.dma_start(wk_sb, wk.rearrange("(c p) n -> p c n", p=128))

        bq_sb = cpool.tile([128, 2], f32, name="bq_sb")
        bk_sb = cpool.tile([128, 2], f32, name="bk_sb")
        nc.sync.dma_start(bq_sb, bq.rearrange("(c p) -> p c", p=128))
        nc.sync.dma_start(bk_sb, bk.rearrange("(c p) -> p c", p=128))

        # x.T streamed in S-chunks of 512: xt[kc][n]
        xt = [[None] * NSQ for _ in range(NK)]

        def load_x_chunk(n):
            for kc in range(NK):
                t = xpool.tile([128, 512], f32r, name=f"xt{kc}_{n}", tag="xt")
                nc.sync.dma_start(
                    t, xT[kc * 128:(kc + 1) * 128, n * 512:(n + 1) * 512])
                xt[kc][n] = t

        load_x_chunk(0)

        wv_sb = wpool.tile([128, NK, F], f32r, name="wv_sb")
        nc.sync.dma_start(wv_sb, wv.rearrange("(c p) n -> p c n", p=128))
        bv_row = cpool.tile([1, F], f32, name="bv_row")
        nc.sync.dma_start(bv_row, bv[None, :])
        bvb = cpool.tile([128, F], f32, name="bvb")
        nc.gpsimd.partition_broadcast(bvb, bv_row)

        load_x_chunk(1)
        wp_sb = wpool.tile([128, 2, D], f32r, name="wp_sb")
        nc.sync.dma_start(wp_sb, wp.rearrange("(c p) n -> p c n", p=128))
        load_x_chunk(2)
        load_x_chunk(3)

        # constants: causal 0/1 triangle, double identity, ones column
        tri = cpool.tile([128, 128], f32, name="tri")
        nc.gpsimd.memset(tri, 1.0)
        nc.gpsimd.affine_select(
            out=tri, in_=tri, compare_op=mybir.AluOpType.is_ge,
            fill=0.0, base=0, pattern=[[1, 128]], channel_multiplier=-1)
        ident = cpool.tile([128, 128], f32, name="ident")
        nc.gpsimd.memset(ident, 0.0)
        nc.gpsimd.affine_select(
            out=ident, in_=ident, compare_op=mybir.AluOpType.not_equal,
            fill=1.0, base=0, pattern=[[-1, 128]], channel_multiplier=1)
        ones_sb = cpool.tile([128, HPC], f32, name="ones_sb")
        nc.gpsimd.memset(ones_sb, 1.0)

        qt = [qkpool.tile([128, S], f32r, name=f"qt{m}", tag="qt") for m in range(2)]
        kt = [qkpool.tile([128, S], f32r, name=f"kt{m}", tag="kt") for m in range(2)]
        at = [apool.tile([128, S], f32r, name=f"at{c}", tag="at") for c in range(2)]
        v65 = [None] * NSK

        # Single S-sweep: QKV for chunk n (PE-heavy), then causal attention
        # for query chunk j=n (ACT-heavy exp), then its output projection.
        # The Tile scheduler overlaps chunk n+1's QKV matmuls with chunk n's
        # softmax, keeping both PE and ACT busy.
        for n in range(NSQ):
            cols = slice(n * 512, (n + 1) * 512)
            for m in range(2):
                psq = psum.tile([128, 512], f32, name="psq", tag="a")
                for kc in range(NK):
                    nc.tensor.matmul(
                        psq, wq_sb[:, kc, m * 128:(m + 1) * 128], xt[kc][n],
                        start=(kc == 0), stop=(kc == NK - 1))
                nc.vector.tensor_scalar_add(qt[m][:, cols], psq,
                                            bq_sb[:, m:m + 1])
                psk = psum.tile([128, 512], f32, name="psk", tag="a")
                for kc in range(NK):
                    nc.tensor.matmul(
                        psk, wk_sb[:, kc, m * 128:(m + 1) * 128], xt[kc][n],
                        start=(kc == 0), stop=(kc == NK - 1))
                nc.vector.tensor_scalar_add(kt[m][:, cols], psk,
                                            bk_sb[:, m:m + 1])
            for sm in range(4 * n, 4 * n + 4):
                scols = slice((sm % 4) * 128, (sm % 4) * 128 + 128)
                rows = slice(sm * 128, (sm + 1) * 128)
                psv = psum.tile([128, F], f32, name="psv", tag="b")
                for kc in range(NK):
                    nc.tensor.matmul(
                        psv, xt[kc][n][:, scols], wv_sb[:, kc, :],
                        start=(kc == 0), stop=(kc == NK - 1))
                vt = vpool.tile([128, HPC, Dh + 1], f32r, name=f"v65_{sm}",
                                tag="v65")
                nc.vector.tensor_copy(vt[:, :, Dh], ones_sb)
                nc.vector.tensor_add(
                    vt[:, :, 0:Dh],
                    psv.rearrange("p (h d) -> p h d", h=HPC),
                    bvb.rearrange("p (h d) -> p h d", h=HPC))
                nc.sync.dma_start(vout[rows, :], vt[:, :, 0:Dh].bitcast(f32))
                v65[sm] = vt

                # k natural via PE transpose of kT (bias already in kT):
                # one [128,128] transpose per channel-chunk (2 heads each)
                kn = knpool.tile([128, F], f32, name="kn", tag="kn")
                for ch2 in range(2):
                    psn = psum.tile([128, 128], f32, name="psn", tag="b")
                    nc.tensor.transpose(
                        psn,
                        kt[ch2][:, sm * 128:(sm + 1) * 128].bitcast(f32),
                        ident)
                    nc.vector.tensor_copy(kn[:, ch2 * 128:(ch2 + 1) * 128], psn)
                nc.sync.dma_start(kout[rows, :], kn)

            # causal attention for query chunk j = n
            j = n
            for h in range(HPC):
                ch = h // 2
                r0 = (h % 2) * 64
                pv = psum.tile([65, 512], f32, name="pv", tag="b")
                ntile = 4 * j + 4
                for i in range(ntile):
                    r = i - 4 * j          # >= 0 on diagonal-block tiles
                    off = 128 * r if r >= 0 else 0
                    qcols = slice(j * 512 + off, (j + 1) * 512)
                    stp = psum.tile([128, 512], f32, name="stp", tag="a")
                    nc.tensor.matmul(
                        stp[:, off:512],
                        kt[ch][r0:r0 + 64, i * 128:(i + 1) * 128],
                        qt[ch][r0:r0 + 64, qcols],
                        start=True, stop=True)
                    ste = stpool.tile([128, 512], f32r, name="ste", tag="ste")
                    nc.scalar.activation(ste[:, off:512], stp[:, off:512],
                                         EXP, scale=0.125)
                    if r >= 0:
                        nc.vector.tensor_mul(ste[:, off:off + 128],
                                             ste[:, off:off + 128], tri)
                    nc.tensor.matmul(
                        pv[:, off:512], v65[i][:, h, :], ste[:, off:512],
                        start=(i == 0), stop=(i == ntile - 1))
                recip = smpool.tile([1, 512], f32, name="recip", tag="recip")
                nc.vector.reciprocal(recip, pv[64:65, :])
                rb = smpool.tile([64, 512], f32, name="rb", tag="rb")
                nc.gpsimd.partition_broadcast(rb, recip)
                nc.vector.tensor_mul(
                    at[ch][r0:r0 + 64, j * 512:(j + 1) * 512], pv[0:64, :], rb)

            # output projection for this quarter of S (partial channels)
            for sm in range(4 * j, 4 * j + 4):
                rows = slice(sm * 128, (sm + 1) * 128)
                for n2 in range(2):
                    ocols = slice(n2 * 512, (n2 + 1) * 512)
                    pr = psum.tile([128, 512], f32, name="pr", tag="b")
                    for cc in range(2):
                        nc.tensor.matmul(
                            pr, at[cc][:, rows], wp_sb[:, cc, ocols],
                            start=(cc == 0), stop=(cc == 1))
                    po = prpool.tile([128, 512], f32, name="po", tag="po")
                    nc.vector.tensor_copy(po, pr)
                    nc.sync.dma_start(outp[rows, ocols], po)


_NC_CACHE = None


def _get_nc():
    global _NC_CACHE
    if _NC_CACHE is None:
        _NC_CACHE = build_nc()
    return _NC_CACHE


def make_in_maps(x, c_attn_w, c_attn_b, c_proj_w, c_proj_b):
    x = np.asarray(x, dtype=np.float32)
    c_attn_w = np.asarray(c_attn_w, dtype=np.float32)
    c_attn_b = np.asarray(c_attn_b, dtype=np.float32)
    c_proj_w = np.asarray(c_proj_w, dtype=np.float32)
    in_maps = []
    for c in range(N_CORES):
        b = c // 4
        g = c % 4
        cs = slice(g * F, (g + 1) * F)          # channel slice for this core
        in_maps.append({
            "xT": round_f32r(x[b].T),
            "wq": round_f32r(c_attn_w[:, 0 * D:1 * D][:, cs]),
            "wk": round_f32r(c_attn_w[:, 1 * D:2 * D][:, cs]),
            "wv": round_f32r(c_attn_w[:, 2 * D:3 * D][:, cs]),
            "wp": round_f32r(c_proj_w[cs, :]),
            "bq": np.ascontiguousarray(c_attn_b[0 * D:1 * D][cs]),
            "bk": np.ascontiguousarray(c_attn_b[1 * D:2 * D][cs]),
            "bv": np.ascontiguousarray(c_attn_b[2 * D:3 * D][cs]),
        })
    return in_maps


def assemble(results, c_proj_b):
    """results: list of 8 dicts with outp/kout/vout -> (a, present)."""
    c_proj_b = np.asarray(c_proj_b, dtype=np.float32)
    B = 2
    a = np.zeros((B, S, D), dtype=np.float32)
    present = np.zeros((B, 2, H, S, Dh), dtype=np.float32)
    for c in range(N_CORES):
        b = c // 4
        g = c % 4
        a[b] += results[c]["outp"]
        hs = slice(g * HPC, (g + 1) * HPC)
        # kout/vout [S, F] -> [S, HPC, Dh] -> [HPC, S, Dh]
        present[b, 0, hs] = results[c]["kout"].reshape(S, HPC, Dh).transpose(1, 0, 2)
        present[b, 1, hs] = results[c]["vout"].reshape(S, HPC, Dh).transpose(1, 0, 2)
    a += c_proj_b[None, None, :]
    return a, present


def kernel(x, c_attn_w, c_attn_b, c_proj_w, c_proj_b):
    global LAST_EXEC_TIME_NS, LAST_RESULTS
    from concourse.bass_utils import run_bass_kernel_spmd

    nc = _get_nc()
    in_maps = make_in_maps(x, c_attn_w, c_attn_b, c_proj_w, c_proj_b)
    res = run_bass_kernel_spmd(nc, in_maps, core_ids=list(range(N_CORES)))
    LAST_EXEC_TIME_NS = res.exec_time_ns
    LAST_RESULTS = res
    return assemble(res.results, c_proj_b)


# revision 18
# speedup vs baseline: 1.3062x; 1.3062x over previous
"""Causal multi-head attention block (QKV proj -> causal attn -> out proj)
for Trainium2, sharded over 8 NeuronCores: data-parallel over batch (B=2),
tensor-parallel over heads (16 heads -> 4 heads per core).

Per-core device program (SPMD, same NEFF, different data):
  inputs : xT [1024,2048] (x[b].T), wq/wk/wv [1024,256], wp [256,1024],
           bq/bk/bv [256]
  outputs: outp [2048,1024] (partial out-proj, host sums 4 cores per batch),
           kout/vout [2048,256] (k,v natural layout for `present`)

Everything on chip is computed in a "transposed" layout so no input
transpose is ever needed on device:
  qT,kT [256,2048] = W.T @ x.T   (Dh on partitions -> scores need no transpose)
  V      [2048,256] = x @ Wv     (natural; used as PV stationary with a fused
                                  ones-column so PV also emits softmax denom)
  scoresT[s_k, s_q] tiles -> exp (no max subtraction; values are bounded)
  attnT  [256,2048] -> out proj natural [2048,1024]
"""

import os
import numpy as np

import concourse.bass as bass
import concourse.bacc as bacc
import concourse.mybir as mybir
import concourse.tile as tile

S = 2048
D = 1024
H = 16
Dh = 64
HPC = 4            # heads per core
F = HPC * Dh       # 256 channels per core
NK = D // 128      # 8 contraction chunks over D
NSK = S // 128     # 16 key tiles
NSQ = S // 512     # 4 query chunks
N_CORES = 8

f32 = mybir.dt.float32
f32r = mybir.dt.float32r
EXP = mybir.ActivationFunctionType.Exp
IDENT = mybir.ActivationFunctionType.Identity

# stash for test harness: exec_time_ns of the last traced run
LAST_EXEC_TIME_NS = None
LAST_RESULTS = None


def round_f32r(a):
    """Round fp32 to the PE's fp32r format (11-bit mantissa, round-half-up
    at bit 12) - matches walrus cast_fp32_to_fp32r."""
    u = np.ascontiguousarray(a, dtype=np.float32).view(np.uint32)
    r = ((u.astype(np.uint64) + 0x800) & 0xFFFFF000).astype(np.uint32)
    return r.view(np.float32)


def build_nc():
    nc = bacc.Bacc("TRN2", target_bir_lowering=False, debug=False)

    xT = nc.dram_tensor("xT", [D, S], f32r, kind="ExternalInput").ap()
    wq = nc.dram_tensor("wq", [D, F], f32r, kind="ExternalInput").ap()
    wk = nc.dram_tensor("wk", [D, F], f32r, kind="ExternalInput").ap()
    wv = nc.dram_tensor("wv", [D, F], f32r, kind="ExternalInput").ap()
    wp = nc.dram_tensor("wp", [F, D], f32r, kind="ExternalInput").ap()
    bq = nc.dram_tensor("bq", [F], f32, kind="ExternalInput").ap()
    bk = nc.dram_tensor("bk", [F], f32, kind="ExternalInput").ap()
    bv = nc.dram_tensor("bv", [F], f32, kind="ExternalInput").ap()

    outp = nc.dram_tensor("outp", [S, D], f32, kind="ExternalOutput").ap()
    kout = nc.dram_tensor("kout", [S, F], f32, kind="ExternalOutput").ap()
    vout = nc.dram_tensor("vout", [S, F], f32, kind="ExternalOutput").ap()

    with tile.TileContext(nc) as tc:
        _body(nc, tc, xT, wq, wk, wv, wp, bq, bk, bv, outp, kout, vout)

    nc.compile()
    return nc


def _body(nc, tc, xT, wq, wk, wv, wp, bq, bk, bv, outp, kout, vout):
    from contextlib import ExitStack

    ctx = ExitStack()
    with ctx:
        wpool = ctx.enter_context(tc.tile_pool(name="wpool", bufs=1))
        xpool = ctx.enter_context(tc.tile_pool(name="xpool", bufs=32))
        qkpool = ctx.enter_context(tc.tile_pool(name="qkpool", bufs=2))
        vpool = ctx.enter_context(tc.tile_pool(name="vpool", bufs=16))
        knpool = ctx.enter_context(tc.tile_pool(name="knpool", bufs=3))
        apool = ctx.enter_context(tc.tile_pool(name="apool", bufs=2))
        stpool = ctx.enter_context(tc.tile_pool(name="stpool", bufs=8))
        prpool = ctx.enter_context(tc.tile_pool(name="prpool", bufs=4))
        smpool = ctx.enter_context(tc.tile_pool(name="smpool", bufs=2))
        cpool = ctx.enter_context(tc.tile_pool(name="cpool", bufs=1))
        psum = ctx.enter_context(tc.tile_pool(name="psum", bufs=4, space="PSUM"))

        # ---- weights for q/k first: QKV starts as soon as x chunk 0 lands
        wq_sb = wpool.tile([128, NK, F], f32r, name="wq_sb")
        wk_sb = wpool.tile([128, NK, F], f32r, name="wk_sb")
        nc.sync.dma_start(wq_sb, wq.rearrange("(c p) n -> p c n", p=128))
        nc.sync.dma_start(wk_sb, wk.rearrange("(c p) n -> p c n", p=128))

        bq_sb = cpool.tile([128, 2], f32, name="bq_sb")
        bk_sb = cpool.tile([128, 2], f32, name="bk_sb")
        nc.sync.dma_start(bq_sb, bq.rearrange("(c p) -> p c", p=128))
        nc.sync.dma_start(bk_sb, bk.rearrange("(c p) -> p c", p=128))

        # x.T streamed in S-chunks of 512: xt[kc][n]
        xt = [[None] * NSQ for _ in range(NK)]

        def load_x_chunk(n):
            for kc in range(NK):
                t = xpool.tile([128, 512], f32r, name=f"xt{kc}_{n}", tag="xt")
                nc.sync.dma_start(
                    t, xT[kc * 128:(kc + 1) * 128, n * 512:(n + 1) * 512])
                xt[kc][n] = t

        load_x_chunk(0)

        wv_sb = wpool.tile([128, NK, F], f32r, name="wv_sb")
        nc.sync.dma_start(wv_sb, wv.rearrange("(c p) n -> p c n", p=128))
        bv_row = cpool.tile([1, F], f32, name="bv_row")
        nc.sync.dma_start(bv_row, bv[None, :])
        bvb = cpool.tile([128, F], f32, name="bvb")
        nc.gpsimd.partition_broadcast(bvb, bv_row)

        load_x_chunk(1)
        wp_sb = wpool.tile([128, 2, D], f32r, name="wp_sb")
        nc.sync.dma_start(wp_sb, wp.rearrange("(c p) n -> p c n", p=128))
        load_x_chunk(2)
        load_x_chunk(3)

        # constants: causal 0/1 triangle, double identity, ones column
        tri = cpool.tile([128, 128], f32, name="tri")
        nc.gpsimd.memset(tri, 1.0)
        nc.gpsimd.affine_select(
            out=tri, in_=tri, compare_op=mybir.AluOpType.is_ge,
            fill=0.0, base=0, pattern=[[1, 128]], channel_multiplier=-1)
        ident = cpool.tile([128, 128], f32, name="ident")
        nc.gpsimd.memset(ident, 0.0)
        nc.gpsimd.affine_select(
            out=ident, in_=ident, compare_op=mybir.AluOpType.not_equal,
            fill=1.0, base=0, pattern=[[-1, 128]], channel_multiplier=1)
        ones_sb = cpool.tile([128, HPC], f32, name="ones_sb")
        nc.gpsimd.memset(ones_sb, 1.0)

        qt = [qkpool.tile([128, S], f32r, name=f"qt{m}", tag="qt") for m in range(2)]
        kt = [qkpool.tile([128, S], f32r, name=f"kt{m}", tag="kt") for m in range(2)]
        at = [apool.tile([128, S], f32r, name=f"at{c}", tag="at") for c in range(2)]
        v65 = [None] * NSK

        # Single S-sweep: QKV for chunk n (PE-heavy), then causal attention
        # for query chunk j=n (ACT-heavy exp), then its output projection.
        # The Tile scheduler overlaps chunk n+1's QKV matmuls with chunk n's
        # softmax, keeping both PE and ACT busy.
        for n in range(NSQ):
            cols = slice(n * 512, (n + 1) * 512)
            for m in range(2):
                psq = psum.tile([128, 512], f32, name="psq", tag="a")
                for kc in range(NK):
                    nc.tensor.matmul(
                        psq, wq_sb[:, kc, m * 128:(m + 1) * 128], xt[kc][n],
                        start=(kc == 0), stop=(kc == NK - 1))
                nc.vector.tensor_scalar_add(qt[m][:, cols], psq,
                                            bq_sb[:, m:m + 1])
                psk = psum.tile([128, 512], f32, name="psk", tag="a")
                for kc in range(NK):
                    nc.tensor.matmul(
                        psk, wk_sb[:, kc, m * 128:(m + 1) * 128], xt[kc][n],
                        start=(kc == 0), stop=(kc == NK - 1))
                nc.vector.tensor_scalar_add(kt[m][:, cols], psk,
                                            bk_sb[:, m:m + 1])
            for sm in range(4 * n, 4 * n + 4):
                scols = slice((sm % 4) * 128, (sm % 4) * 128 + 128)
                rows = slice(sm * 128, (sm + 1) * 128)
                psv = psum.tile([128, F], f32, name="psv", tag="b")
                for kc in range(NK):
                    nc.tensor.matmul(
                        psv, xt[kc][n][:, scols], wv_sb[:, kc, :],
                        start=(kc == 0), stop=(kc == NK - 1))
                vt = vpool.tile([128, HPC, Dh + 1], f32r, name=f"v65_{sm}",
                                tag="v65")
                nc.vector.tensor_copy(vt[:, :, Dh], ones_sb)
                nc.vector.tensor_add(
                    vt[:, :, 0:Dh],
                    psv.rearrange("p (h d) -> p h d", h=HPC),
                    bvb.rearrange("p (h d) -> p h d", h=HPC))
                nc.sync.dma_start(vout[rows, :], vt[:, :, 0:Dh].bitcast(f32))
                v65[sm] = vt

                # k natural via PE transpose of kT (bias already in kT):
                # one [128,128] transpose per channel-chunk (2 heads each)
                kn = knpool.tile([128, F], f32, name="kn", tag="kn")
                for ch2 in range(2):
                    psn = psum.tile([128, 128], f32, name="psn", tag="b")
                    nc.tensor.transpose(
                        psn,
                        kt[ch2][:, sm * 128:(sm + 1) * 128].bitcast(f32),
                        ident)
                    nc.vector.tensor_copy(kn[:, ch2 * 128:(ch2 + 1) * 128], psn)
                nc.sync.dma_start(kout[rows, :], kn)

            # causal attention for query chunk j = n
            j = n
            for h in range(HPC):
                ch = h // 2
                r0 = (h % 2) * 64
                pv = psum.tile([65, 512], f32, name="pv", tag="b")
                ntile = 4 * j + 4
                for i in range(ntile):
                    r = i - 4 * j          # >= 0 on diagonal-block tiles
                    off = 128 * r if r >= 0 else 0
                    qcols = slice(j * 512 + off, (j + 1) * 512)
                    stp = psum.tile([128, 512], f32, name="stp", tag="a")
                    nc.tensor.matmul(
                        stp[:, off:512],
                        kt[ch][r0:r0 + 64, i * 128:(i + 1) * 128],
                        qt[ch][r0:r0 + 64, qcols],
                        start=True, stop=True)
                    ste = stpool.tile([128, 512], f32r, name="ste", tag="ste")
                    nc.scalar.activation(ste[:, off:512], stp[:, off:512],
                                         EXP, scale=0.125)
                    if r >= 0:
                        nc.vector.tensor_mul(ste[:, off:off + 128],
                                             ste[:, off:off + 128], tri)
                    nc.tensor.matmul(
                        pv[:, off:512], v65[i][:, h, :], ste[:, off:512],
                        start=(i == 0), stop=(i == ntile - 1))
                recip = smpool.tile([1, 512], f32, name="recip", tag="recip")
                nc.vector.reciprocal(recip, pv[64:65, :])
                rb = smpool.tile([64, 512], f32, name="rb", tag="rb")
                nc.gpsimd.partition_broadcast(rb, recip)
                nc.vector.tensor_mul(
                    at[ch][r0:r0 + 64, j * 512:(j + 1) * 512], pv[0:64, :], rb)

            # output projection for this quarter of S (partial channels)
            for sm in range(4 * j, 4 * j + 4):
                rows = slice(sm * 128, (sm + 1) * 128)
                for n2 in range(2):
                    ocols = slice(n2 * 512, (n2 + 1) * 512)
                    pr = psum.tile([128, 512], f32, name="pr", tag="b")
                    for cc in range(2):
                        nc.tensor.matmul(
                            pr, at[cc][:, rows], wp_sb[:, cc, ocols],
                            start=(cc == 0), stop=(cc == 1))
                    po = prpool.tile([128, 512], f32, name="po", tag="po")
                    nc.vector.tensor_copy(po, pr)
                    nc.sync.dma_start(outp[rows, ocols], po)


_NC_CACHE = None


def _get_nc():
    global _NC_CACHE
    if _NC_CACHE is None:
        _NC_CACHE = build_nc()
    return _NC_CACHE


def make_in_maps(x, c_attn_w, c_attn_b, c_proj_w, c_proj_b):
    x = np.asarray(x, dtype=np.float32)
    c_attn_w = np.asarray(c_attn_w, dtype=np.float32)
    c_attn_b = np.asarray(c_attn_b, dtype=np.float32)
    c_proj_w = np.asarray(c_proj_w, dtype=np.float32)
    in_maps = []
    for c in range(N_CORES):
        b = c // 4
        g = c % 4
        cs = slice(g * F, (g + 1) * F)          # channel slice for this core
        in_maps.append({
            "xT": round_f32r(x[b].T),
            "wq": round_f32r(c_attn_w[:, 0 * D:1 * D][:, cs]),
            "wk": round_f32r(c_attn_w[:, 1 * D:2 * D][:, cs]),
            "wv": round_f32r(c_attn_w[:, 2 * D:3 * D][:, cs]),
            "wp": round_f32r(c_proj_w[cs, :]),
            "bq": np.ascontiguousarray(c_attn_b[0 * D:1 * D][cs]),
            "bk": np.ascontiguousarray(c_attn_b[1 * D:2 * D][cs]),
            "bv": np.ascontiguousarray(c_attn_b[2 * D:3 * D][cs]),
        })
    return in_maps


def assemble(results, c_proj_b):
    """results: list of 8 dicts with outp/kout/vout -> (a, present)."""
    c_proj_b = np.asarray(c_proj_b, dtype=np.float32)
    B = 2
    a = np.zeros((B, S, D), dtype=np.float32)
    present = np.zeros((B, 2, H, S, Dh), dtype=np.float32)
    for c in range(N_CORES):
        b = c // 4
        g = c % 4
        a[b] += results[c]["outp"]
        hs = slice(g * HPC, (g + 1) * HPC)
        # kout/vout [S, F] -> [S, HPC, Dh] -> [HPC, S, Dh]
        present[b, 0, hs] = results[c]["kout"].reshape(S, HPC, Dh).transpose(1, 0, 2)
        present[b, 1, hs] = results[c]["vout"].reshape(S, HPC, Dh).transpose(1, 0, 2)
    a += c_proj_b[None, None, :]
    return a, present


def kernel(x, c_attn_w, c_attn_b, c_proj_w, c_proj_b):
    global LAST_EXEC_TIME_NS, LAST_RESULTS
    from concourse.bass_utils import run_bass_kernel_spmd

    nc = _get_nc()
    in_maps = make_in_maps(x, c_attn_w, c_attn_b, c_proj_w, c_proj_b)
    res = run_bass_kernel_spmd(nc, in_maps, core_ids=list(range(N_CORES)))
    LAST_EXEC_TIME_NS = res.exec_time_ns
    LAST_RESULTS = res
    return assemble(res.results, c_proj_b)
